# revision 27
# baseline (speedup 1.0000x reference)
"""Trainium2 Bass kernel for nn_DecoderRoPEBlock (B=4, LQ=1024, LC=512,
E=1024, H=16, FF=4096) running SPMD on 8 NeuronCores.

Sharding: 8 cores = (batch, striped query-tiles); zero collectives.
Stage-1 causal self-attention K/V are recomputed per core from the
original x, so each core produces its 512 output rows independently.
Causal striping: each core owns interleaved 128-token query tiles.

fp8 path: every matmul (projections, fc1/fc2, scores, AV, LN stats)
consumes fp8e4 operands in MatmulPerfMode.DoubleRow (two contraction
rows per PE pass, 2x throughput).  Weights are scaled by 2^10; the
dequant folds into the softmax exp scale (q,k), the V psum->sbuf copy,
the gelu activation scale (fc1), and the host-side gate vector.

Q/K layout for DoubleRow scores: heads are stored 4-per-tile, 32
partitions each, with the rope hd-halves (x1/x2) as the fp8 pair dim:
tile[32*hl+p, i, l] = rope half i of head 4G+hl, freq p.  The rotation
is applied by full-width vector ops reading the projection PSUM
directly (r1 = A*cos - B*sin, r2 = A*sin + B*cos), so no partition-swap
DMAs and no psum->sbuf staging copies are needed.
"""
import sys
sys.path.insert(0, '/opt/trn_rl_repo')
from contextlib import ExitStack

import numpy as np
import ml_dtypes

import concourse.bass as bass
from concourse import bacc
import concourse.tile as tile
import concourse.mybir as mybir

f32 = mybir.dt.float32
bf16 = mybir.dt.bfloat16
f8 = mybir.dt.float8e4
AF = mybir.ActivationFunctionType
ALU = mybir.AluOpType
DR = mybir.MatmulPerfMode.DoubleRow
EPS = 1e-6
P = 128
SW = 1024.0          # weight fp8 scale (power of two)
ISW = 1.0 / SW


class Cfg:
    def __init__(self, E, H, LQ, LC, B, FF, n_cores):
        self.E, self.H, self.LQ, self.LC, self.B, self.FF = E, H, LQ, LC, B, FF
        self.HD = E // H
        assert self.HD == 64, "rope layout assumes head dim 64"
        self.n_cores = n_cores
        self.qsplit = n_cores // B
        assert B * self.qsplit == n_cores
        self.Lq = LQ // self.qsplit
        assert self.Lq <= 512
        self.Lk = LQ
        self.Lc = LC
        self.nec = E // P
        self.ng = self.nec // 2          # fp8 pair groups over E
        self.nkt = self.Lk // P
        self.nct = self.Lc // P
        self.npr = H // 2
        self.nhg = H // 4                # 4-head groups (q/k layout)
        self.nft = FF // P
        self.NT = 512


def _steer_act_tables(arch):
    """Keep Ln/Exp/Square together in natural_log_exp_and_others so the
    LN rstd pipeline (Ln -> Exp) and softmax Exp never reload tables."""
    from concourse.hw_specs import get_activation_tables
    try:
        tabs = get_activation_tables(arch)
    except Exception:
        return
    target = 'natural_log_exp_and_others'
    if target not in tabs:
        return
    keep = tabs[target]
    for name, s in tabs.items():
        if name == target:
            continue
        if AF.Exp in keep:
            s.discard(AF.Exp)
        if AF.Ln in keep:
            s.discard(AF.Ln)
        if AF.Square in keep:
            s.discard(AF.Square)


def build_core_program(cfg: Cfg):
    c = cfg
    nc = bacc.Bacc()
    _steer_act_tables(nc.m.arch)

    d_xT = nc.declare_dram_parameter("xT", [c.E, c.Lq], f32, isOutput=False)
    d_x8 = nc.declare_dram_parameter("x8", [P, c.nec, c.Lk], f8, isOutput=False)
    d_ctx8 = nc.declare_dram_parameter("ctx8", [P, c.nec, c.Lc], f8,
                                       isOutput=False)
    d_mask = nc.declare_dram_parameter("mask8", [c.Lk, P], f8, isOutput=False)
    WNAMES = ["sa_q", "sa_k", "sa_v", "sa_p", "ca_q", "ca_k", "ca_v", "ca_p"]
    d_w = {n: nc.declare_dram_parameter("w_" + n, [P, c.nec, c.E], f8,
                                        isOutput=False)
           for n in WNAMES}
    d_fc1 = nc.declare_dram_parameter("w_fc1", [P, c.nec, c.FF], f8,
                                      isOutput=False)
    d_fc2 = nc.declare_dram_parameter("w_fc2", [P, c.nft, c.E], f8,
                                      isOutput=False)
    d_cq = nc.declare_dram_parameter("cos_q", [P, c.Lq], bf16, isOutput=False)
    d_sq = nc.declare_dram_parameter("sin_q", [P, c.Lq], bf16, isOutput=False)
    d_ck = nc.declare_dram_parameter("cos_k", [P, c.Lk], bf16, isOutput=False)
    d_sk = nc.declare_dram_parameter("sin_k", [P, c.Lk], bf16, isOutput=False)
    d_cc = nc.declare_dram_parameter("cos_c", [P, c.Lc], bf16, isOutput=False)
    d_sc = nc.declare_dram_parameter("sin_c", [P, c.Lc], bf16, isOutput=False)
    d_g = nc.declare_dram_parameter("gvec", [c.E, 3], f32, isOutput=False)
    d_out = nc.declare_dram_parameter("outT", [c.E, c.Lq], f32, isOutput=True)

    Lq, Lk, Lc, nec, ng, nkt, nct, npr, nhg, nft = (
        c.Lq, c.Lk, c.Lc, c.nec, c.ng, c.nkt, c.nct, c.npr, c.nhg, c.nft)
    VNT = min(c.NT, c.E)
    n_vnt = c.E // VNT
    KNT = min(c.NT, Lk)
    n_knt = Lk // KNT
    # q,k weight fp8 scales are folded into the host cos/sin tables, so
    # the roped q/k land at natural magnitude (fp8-safe) and the softmax
    # scale stays 1/sqrt(HD).
    EXP_SC = 0.125

    with tile.TileContext(nc) as tc, ExitStack() as ctx:
        # -------------------- pools --------------------
        p_x = ctx.enter_context(tc.tile_pool(name="p_x", bufs=1))
        p_h = ctx.enter_context(tc.tile_pool(name="p_h", bufs=1))
        p_s1k = ctx.enter_context(tc.tile_pool(name="p_s1k", bufs=28))
        p_w = ctx.enter_context(tc.tile_pool(name="p_w", bufs=2))
        p_per = ctx.enter_context(tc.tile_pool(name="p_per", bufs=1))
        p_scr = ctx.enter_context(tc.tile_pool(name="p_scr", bufs=1))
        p_sm = ctx.enter_context(tc.tile_pool(name="p_sm", bufs=1))
        p_t = ctx.enter_context(tc.tile_pool(name="p_t", bufs=3))
        ps_mm = ctx.enter_context(tc.tile_pool(name="ps_mm", bufs=2, space="PSUM"))
        ps_av = ctx.enter_context(tc.tile_pool(name="ps_av", bufs=1, space="PSUM"))
        ps_st = ctx.enter_context(tc.tile_pool(name="ps_st", bufs=1, space="PSUM"))
        ps_bc = ctx.enter_context(tc.tile_pool(name="ps_bc", bufs=2, space="PSUM"))

        # -------------------- prologue loads --------------------
        xT = []
        for e in range(nec):
            t = p_x.tile([P, Lq], f32, tag=f"x{e}", name=f"x{e}")
            nc.sync.dma_start(out=t[:], in_=d_xT[e * P:(e + 1) * P, :])
            xT.append(t)
        x8 = []
        for g in range(ng):
            t = p_h.tile([P, 2, Lk], f8, tag=f"h{g}", name=f"h{g}")
            nc.sync.dma_start(out=t[:], in_=d_x8[:, 2 * g:2 * g + 2, :])
            x8.append(t)
        masks = []
        for kt in range(nkt):
            t = p_per.tile([P, P], f8, tag=f"mask{kt}", name=f"mask{kt}")
            nc.sync.dma_start(out=t[:], in_=d_mask[kt * P:(kt + 1) * P, :])
            masks.append(t)
        cq = p_per.tile([P, Lq], bf16, tag="cq", name="cq")
        sq = p_per.tile([P, Lq], bf16, tag="sq", name="sq")
        ck = p_per.tile([P, Lk], bf16, tag="ck", name="ck")
        sk = p_per.tile([P, Lk], bf16, tag="sk", name="sk")
        ccos = p_per.tile([P, Lc], bf16, tag="ccos", name="ccos")
        csin = p_per.tile([P, Lc], bf16, tag="csin", name="csin")
        for t, d in ((cq, d_cq), (sq, d_sq), (ck, d_ck), (sk, d_sk),
                     (ccos, d_cc), (csin, d_sc)):
            nc.sync.dma_start(out=t[:], in_=d[:, :])
        gsb = p_per.tile([P, nec, 3], f32, tag="g", name="g")
        for e in range(nec):
            nc.sync.dma_start(out=gsb[:, e, :], in_=d_g[e * P:(e + 1) * P, :])
        ones_bc = p_per.tile([P, 2, P], f8, tag="ones_bc", name="ones_bc")
        nc.vector.memset(ones_bc[:], 1.0)
        epsb = p_per.tile([P, 1], f32, tag="epsb", name="epsb")
        nc.vector.memset(epsb[:], EPS)

        def load_w8(dram, tag="wproj"):
            t = p_w.tile([P, nec, c.E], f8, tag=tag, name=tag)
            for g in range(ng):
                nc.sync.dma_start(out=t[:, 2 * g:2 * g + 2, :],
                                  in_=dram[:, 2 * g:2 * g + 2, :])
            return t

        # ==================== LN ====================
        def layer_norm(src_pairs, L, sq_tag, src_f32=None):
            """LN over E of fp8-paired src [P, 2, L] x ng -> fp8 h pairs."""
            n_lt = max(1, L // 512)
            LT = L // n_lt
            # phase A: stats broadcast across partitions by ones[P,2,128]
            rstds, ccvs = [], []
            for lt in range(n_lt):
                sl = slice(lt * LT, (lt + 1) * LT)
                sq_t = []
                for g in range(ng):
                    s = p_scr.tile([P, 2, LT], f8, tag=f"{sq_tag}{g}",
                                   name=f"{sq_tag}{g}")
                    nc.vector.tensor_mul(s[:], src_pairs[g][:, :, sl],
                                         src_pairs[g][:, :, sl])
                    sq_t.append(s)
                s1 = ps_st.tile([P, LT], f32, tag="s1", name="s1")
                s2 = ps_st.tile([P, LT], f32, tag="s2", name="s2")
                for g in range(ng):
                    nc.tensor.matmul(s1[:], ones_bc[:], src_pairs[g][:, :, sl],
                                     start=(g == 0), stop=(g == ng - 1),
                                     perf_mode=DR)
                for g in range(ng):
                    nc.tensor.matmul(s2[:], ones_bc[:], sq_t[g][:],
                                     start=(g == 0), stop=(g == ng - 1),
                                     perf_mode=DR)
                mu = p_sm.tile([P, LT], f32, tag="lnsc", name="mu", bufs=4)
                nc.scalar.mul(mu[:], s1[:], 1.0 / c.E)
                mu2 = p_sm.tile([P, LT], f32, tag="lnsc", name="mu2", bufs=4)
                nc.scalar.square(mu2[:], mu[:])
                s2c = p_sm.tile([P, LT], f32, tag="lnsc", name="s2c", bufs=4)
                nc.scalar.mul(s2c[:], s2[:], 1.0 / c.E)
                var = p_sm.tile([P, LT], f32, tag="lnsc", name="var", bufs=4)
                nc.vector.tensor_sub(var[:], s2c[:], mu2[:])
                lnv = p_sm.tile([P, LT], f32, tag="lnsc", name="lnv", bufs=4)
                nc.scalar.activation(out=lnv[:], in_=var[:], func=AF.Ln,
                                     bias=epsb[:])
                rstd = p_sm.tile([P, LT], bf16, tag="rstd", name="rstd", bufs=2)
                nc.scalar.activation(out=rstd[:], in_=lnv[:], func=AF.Exp,
                                     scale=-0.5)
                ccv = p_sm.tile([P, LT], bf16, tag="ccv", name="ccv", bufs=2)
                nc.vector.tensor_mul(ccv[:], mu[:], rstd[:])
                rstds.append(rstd)
                ccvs.append(ccv)
            # phase C: apply; fully read src pair tile before writing h pair
            # (h pair tiles alias src pair tiles via tag reuse in stage 1)
            hs = [p_h.tile([P, 2, L], f8, tag=f"h{g}", name=f"hln{g}")
                  for g in range(ng)]
            for g in range(ng):
                tmps = []
                for half in range(2):
                    e = 2 * g + half
                    src = (src_f32[e] if src_f32 is not None
                           else src_pairs[g][:, half, :])
                    for lt in range(n_lt):
                        sl = slice(lt * LT, (lt + 1) * LT)
                        tmp = p_t.tile([P, LT], bf16, tag="lntmp",
                                       name="lntmp", bufs=4)
                        nc.vector.tensor_mul(tmp[:], src[:, sl], rstds[lt][:])
                        tmps.append((half, lt, tmp))
                for half, lt, tmp in tmps:
                    sl = slice(lt * LT, (lt + 1) * LT)
                    nc.gpsimd.tensor_sub(hs[g][:, half, sl], tmp[:],
                                         ccvs[lt][:])
            return hs

        # ==================== q/k projection + rope ====================
        def qk_project_rope(w8, rhs_pairs, dst_tiles, dst_sl, L,
                            cos_t, sin_t, rhs_lslice=None):
            """Project and rotate into 4-head pair layout.

            dst_tiles[G][32*hl+p, 0, l] = A*cos - B*sin
            dst_tiles[G][32*hl+p, 1, l] = A*sin + B*cos
            where A/B are the even/odd rope halves produced as psum
            chunks 2G / 2G+1 by the host-permuted weights."""
            cs = cos_t[:, dst_sl]
            sn = sin_t[:, dst_sl]
            for G in range(nhg):
                pss = []
                for half in range(2):
                    eo = 2 * G + half
                    ps = ps_mm.tile([P, L], f32, tag="mm", name="mm")
                    for g in range(ng):
                        r = (rhs_pairs[g][:] if rhs_lslice is None
                             else rhs_pairs[g][:, :, rhs_lslice])
                        nc.tensor.matmul(
                            ps[:], w8[:, 2 * g:2 * g + 2, eo * P:(eo + 1) * P],
                            r, start=(g == 0), stop=(g == ng - 1),
                            perf_mode=DR)
                    pss.append(ps)
                psA, psB = pss
                t1 = p_t.tile([P, L], bf16, tag="ropet1", name="ropet1", bufs=2)
                nc.vector.tensor_mul(t1[:], psA[:], cs)
                t2 = p_t.tile([P, L], bf16, tag="ropet2", name="ropet2", bufs=2)
                nc.vector.tensor_mul(t2[:], psB[:], sn)
                nc.vector.tensor_sub(dst_tiles[G][:, 0, dst_sl], t1[:], t2[:])
                t3 = p_t.tile([P, L], bf16, tag="ropet3", name="ropet3", bufs=2)
                nc.vector.tensor_mul(t3[:], psA[:], sn)
                t4 = p_t.tile([P, L], bf16, tag="ropet4", name="ropet4", bufs=2)
                nc.vector.tensor_mul(t4[:], psB[:], cs)
                nc.vector.tensor_add(dst_tiles[G][:, 1, dst_sl], t3[:], t4[:])

        # ==================== V projection ====================
        def v_project8(w8, rhs_pairs, n_kt):
            """V pairs over key tiles: [tok, i, H, 65] with ones column."""
            v_pairs = []
            for t in range(n_kt // 2):
                vt = p_per.tile([P, 2, c.H, 65], f8, tag=f"v{t}", name=f"v{t}")
                nc.vector.memset(vt[:, :, :, 64:65], 1.0)
                v_pairs.append(vt)
            for kt in range(n_kt):
                tok = slice(kt * P, (kt + 1) * P)
                for vn in range(n_vnt):
                    ps = ps_mm.tile([P, VNT], f32, tag="mm", name="mm")
                    for g in range(ng):
                        nc.tensor.matmul(
                            ps[:],
                            rhs_pairs[g][:, :, tok],
                            w8[:, 2 * g:2 * g + 2, vn * VNT:(vn + 1) * VNT],
                            start=(g == 0), stop=(g == ng - 1),
                            perf_mode=DR)
                    nh = VNT // 64
                    nc.scalar.mul(
                        v_pairs[kt // 2][:, kt % 2, vn * nh:(vn + 1) * nh, 0:64],
                        ps[:].rearrange("p (nh d) -> p nh d", d=64), ISW)
            return v_pairs

        # ==================== attention ====================
        def attention(qp, kp, v_pairs, n_kt, use_mask, su_list=None):
            """q/k in 4-head pair layout; fp8 DoubleRow scores and AV.
            su_list[kt] = first useful q column for key tile kt (causal
            striping).  Returns fp8 Onorm pair tiles."""
            if su_list is None:
                su_list = [0] * n_kt
            n_t = n_kt // 2
            on_pairs = [p_per.tile([P, 2, Lq], f8, tag=f"on{j}", name=f"on{j}")
                        for j in range(ng)]
            for pr in range(npr):
                G = pr // 2
                o_pair = ps_av.tile([65, 2 * Lq], f32, tag="av", name="o_pair")
                results = []
                for hh, qoff in ((2 * pr, 0), (2 * pr + 1, Lq)):
                    hl = hh % 4
                    pb = 32 * hl
                    ex_pairs = []
                    score_roster = [(ps_mm, "mm"), (ps_mm, "mm"),
                                    (ps_st, "s1"), (ps_st, "s2")]
                    for t in range(n_t):
                        ex = p_s1k.tile([P, 2, Lq], f8, tag="s1k", name="ex")
                        for half in range(2):
                            kt = 2 * t + half
                            su = su_list[kt]
                            pl, tg = score_roster[kt % 4]
                            s_ps = pl.tile([P, Lq], f32, tag=tg, name="s_ps")
                            nc.tensor.matmul(
                                s_ps[:, su:],
                                kp[G][pb:pb + 32, :, kt * P:(kt + 1) * P],
                                qp[G][pb:pb + 32, :, su:],
                                start=True, stop=True, perf_mode=DR,
                                tile_position=(pb, 0))
                            nc.scalar.activation(out=ex[:, half, su:],
                                                 in_=s_ps[:, su:],
                                                 func=AF.Exp, scale=EXP_SC)
                            if use_mask:
                                nc.gpsimd.tensor_mul(
                                    ex[:, half, su:su + P],
                                    ex[:, half, su:su + P],
                                    masks[kt][:, 0:P])
                        su0, su1 = su_list[2 * t], su_list[2 * t + 1]
                        if su1 > su0:
                            nc.vector.memset(ex[:, 1, su0:su1], 0.0)
                        ex_pairs.append(ex)
                    for t in range(n_t):
                        su0 = min(su_list[2 * t], su_list[2 * t + 1])
                        nc.tensor.matmul(o_pair[:, qoff + su0:qoff + Lq],
                                         v_pairs[t][:, :, hh, :],
                                         ex_pairs[t][:, :, su0:],
                                         start=(t == 0), stop=(t == n_t - 1),
                                         perf_mode=DR)
                    results.append((64 * (hh % 2), qoff))
                lnd = p_sm.tile([1, 2 * Lq], bf16, tag="rec", name="lnd", bufs=2)
                nc.scalar.activation(out=lnd[:], in_=o_pair[64:65, :],
                                     func=AF.Ln)
                rec = p_sm.tile([1, 2 * Lq], bf16, tag="rec", name="rec", bufs=2)
                nc.scalar.activation(out=rec[:], in_=lnd[:],
                                     func=AF.Exp, scale=-1.0)
                on = on_pairs[pr // 2]
                half = pr % 2
                for pbase, qoff in results:
                    db = p_t.tile([64, Lq], bf16, tag="db", name="db", bufs=2)
                    nc.gpsimd.partition_broadcast(
                        db[:], rec[0:1, qoff:qoff + Lq], channels=64)
                    nc.vector.scalar_tensor_tensor(
                        out=on[pbase:pbase + 64, half, :],
                        in0=o_pair[0:64, qoff:qoff + Lq], scalar=1.0, in1=db[:],
                        op0=ALU.bypass, op1=ALU.mult)
            return on_pairs

        def proj_residual8(w8, src_pairs, g_idx):
            for e in range(nec):
                ps = ps_mm.tile([P, Lq], f32, tag="mm", name="mm")
                for g in range(ng):
                    nc.tensor.matmul(ps[:],
                                     w8[:, 2 * g:2 * g + 2, e * P:(e + 1) * P],
                                     src_pairs[g][:],
                                     start=(g == 0), stop=(g == ng - 1),
                                     perf_mode=DR)
                nc.vector.scalar_tensor_tensor(
                    out=xT[e][:], in0=ps[:], scalar=gsb[:, e, g_idx:g_idx + 1],
                    in1=xT[e][:], op0=ALU.mult, op1=ALU.add)

        # ==================== STAGE 1: causal self-attention ============
        h1 = layer_norm(x8, Lk, "scrA")

        qp1 = [p_per.tile([P, 2, Lq], f8, tag=f"qp{G}", name=f"qp{G}")
               for G in range(nhg)]
        kp1 = [p_per.tile([P, 2, Lk], f8, tag=f"kp{G}", name=f"kp{G}")
               for G in range(nhg)]
        w = load_w8(d_w["sa_q"])
        qk_project_rope(w, h1, qp1, slice(0, Lq), Lq, cq, sq,
                        rhs_lslice=slice(0, Lq))
        w = load_w8(d_w["sa_k"])
        for nt in range(n_knt):
            sl = slice(nt * KNT, (nt + 1) * KNT)
            qk_project_rope(w, h1, kp1, sl, KNT, ck, sk, rhs_lslice=sl)
        w = load_w8(d_w["sa_v"])
        v1 = v_project8(w, h1, nkt)
        su_sa = [min(P * (kt % (nkt // 2)), Lq - P) for kt in range(nkt)]
        on1 = attention(qp1, kp1, v1, nkt, True, su_list=su_sa)
        w = load_w8(d_w["sa_p"])
        proj_residual8(w, on1, 0)

        # ==================== STAGE 2: cross-attention ==================
        x8_2 = []
        for g in range(ng):
            t = p_scr.tile([P, 2, Lq], f8, tag=f"scrB{g}", name=f"scrB{g}")
            nc.gpsimd.tensor_copy(t[:, 0, :], xT[2 * g][:])
            nc.gpsimd.tensor_copy(t[:, 1, :], xT[2 * g + 1][:])
            x8_2.append(t)
        h2 = layer_norm(x8_2, Lq, "scrA", src_f32=xT)
        ctx8 = []
        for g in range(ng):
            t = p_per.tile([P, 2, Lc], f8, tag=f"ctx{g}", name=f"ctx{g}")
            nc.sync.dma_start(out=t[:], in_=d_ctx8[:, 2 * g:2 * g + 2, :])
            ctx8.append(t)
        qp2 = [p_per.tile([P, 2, Lq], f8, tag=f"qp{G}", name=f"qp2_{G}")
               for G in range(nhg)]
        kp2 = [p_per.tile([P, 2, Lc], f8, tag=f"kp{G}", name=f"kp2_{G}")
               for G in range(nhg)]
        w = load_w8(d_w["ca_q"])
        qk_project_rope(w, h2, qp2, slice(0, Lq), Lq, cq, sq)
        w = load_w8(d_w["ca_k"])
        qk_project_rope(w, ctx8, kp2, slice(0, Lc), Lc, ccos, csin)
        w = load_w8(d_w["ca_v"])
        v2 = v_project8(w, ctx8, nct)
        on2 = attention(qp2, kp2, v2, nct, False)
        w = load_w8(d_w["ca_p"])
        proj_residual8(w, on2, 1)

        # ==================== STAGE 3: MLP ==============================
        x8_3 = []
        for g in range(ng):
            t = p_scr.tile([P, 2, Lq], f8, tag=f"scrB{g}", name=f"scrB{g}")
            nc.gpsimd.tensor_copy(t[:, 0, :], xT[2 * g][:])
            nc.gpsimd.tensor_copy(t[:, 1, :], xT[2 * g + 1][:])
            x8_3.append(t)
        h3 = layer_norm(x8_3, Lq, "scrA", src_f32=xT)
        a_pairs = []
        for fi in range(nft):
            wt = p_w.tile([P, nec, P], f8, tag="wstream", name="wf1", bufs=6)
            nc.sync.dma_start(out=wt[:], in_=d_fc1[:, :, fi * P:(fi + 1) * P])
            ps = ps_mm.tile([P, Lq], f32, tag="mm", name="mm")
            for g in range(ng):
                nc.tensor.matmul(ps[:], wt[:, 2 * g:2 * g + 2, :],
                                 h3[g][:], start=(g == 0),
                                 stop=(g == ng - 1), perf_mode=DR)
            if fi % 2 == 0:
                ap = p_s1k.tile([P, 2, Lq], f8, tag="s1k", name="apair")
                a_pairs.append(ap)
            nc.scalar.activation(out=a_pairs[fi // 2][:, fi % 2, :],
                                 in_=ps[:], func=AF.Gelu_apprx_tanh,
                                 scale=ISW)
        # fc2: occupy all 8 psum banks as accumulators, stream fc2 weights
        acc_pools = [ps_mm, ps_mm, ps_st, ps_st, ps_bc, ps_bc]
        acc_tags = ["mm", "mm", "s1", "s2", "bc", "bc"]
        accs = []
        for e in range(min(nec, 6)):
            pl = acc_pools[e]
            accs.append(pl.tile([P, Lq], f32, tag=acc_tags[e],
                                name=f"fc2acc{e}"))
        if nec > 6:
            acc_pair = ps_av.tile([P, 2 * Lq], f32, tag="av", name="fc2accp")
            accs.append(acc_pair[:, 0:Lq])
            accs.append(acc_pair[:, Lq:2 * Lq])
        nfp = nft // 2
        for t2 in range(nfp):
            wt = p_w.tile([P, 2, c.E], f8, tag="wstream", name="wfc2", bufs=6)
            nc.sync.dma_start(out=wt[:], in_=d_fc2[:, 2 * t2:2 * t2 + 2, :])
            for e in range(nec):
                nc.tensor.matmul(accs[e][:], wt[:, :, e * P:(e + 1) * P],
                                 a_pairs[t2][:], start=(t2 == 0),
                                 stop=(t2 == nfp - 1), perf_mode=DR)
        for e in range(nec):
            nc.vector.scalar_tensor_tensor(
                out=xT[e][:], in0=accs[e][:], scalar=gsb[:, e, 2:3],
                in1=xT[e][:], op0=ALU.mult, op1=ALU.add)

        # ==================== output ====================
        for e in range(nec):
            nc.sync.dma_start(out=d_out[e * P:(e + 1) * P, :], in_=xT[e][:])

    nc.finalize()
    return nc


# ======================================================================
# Host-side preparation
# ======================================================================
def rope_tables32(positions, HD, dtype=np.float32):
    """[128, L] tables for the 4-head/32-partition pair layout: row
    32*hl + p holds cos/sin(pos * inv_freq[p]) / SW.  The 1/SW undoes
    the fp8 weight scale so roped q/k stay in fp8 range."""
    inv_freq = 1.0 / (10000.0 ** (np.arange(0, HD, 2, dtype=np.float64) / HD))
    ang = positions[None, :].astype(np.float64) * inv_freq[:, None]  # [32, L]
    cos, sin = np.cos(ang) * ISW, np.sin(ang) * ISW
    return (np.tile(cos, (4, 1)).astype(dtype),
            np.tile(sin, (4, 1)).astype(dtype))


def rope_perm2(E, HD):
    """Output-row permutation: per 4-head group, the 128 even (x1) dims
    of the 4 heads, then the 128 odd (x2) dims."""
    H = E // HD
    idx = []
    for G in range(H // 4):
        for h in range(4 * G, 4 * G + 4):
            idx.extend(h * HD + np.arange(0, HD, 2))
        for h in range(4 * G, 4 * G + 4):
            idx.extend(h * HD + np.arange(1, HD, 2))
    return np.asarray(idx, dtype=np.int64)


def to_bf(a):
    return np.asarray(a, dtype=np.float32).astype(ml_dtypes.bfloat16)


def to_f8(a):
    return np.clip(np.asarray(a, np.float32), -224.0, 224.0).astype(
        ml_dtypes.float8_e4m3)


def pack_pairs(W):
    """[K, M] -> [128, K//128, M] with K-chunk index in the middle."""
    K, M = W.shape
    return np.ascontiguousarray(
        W.reshape(K // 128, 128, M).transpose(1, 0, 2))


def host_prep(inputs, cfg: Cfg):
    c = cfg
    E, HD = c.E, c.HD
    perm = rope_perm2(E, HD)

    def ln_fold(w, nw, do_perm):
        weff = np.asarray(w, np.float64)
        if nw is not None:
            weff = weff * np.asarray(nw, np.float64)[None, :]
        if do_perm:
            weff = weff[perm, :]
        return weff.T

    def w8(weff):
        return to_f8(pack_pairs(np.asarray(weff, np.float32) * SW))

    x = np.asarray(inputs['x'], np.float32)
    ctxv = np.asarray(inputs['context'], np.float32)
    am = np.asarray(inputs['attn_mask'])
    n1w, n2w, n3w = (np.asarray(inputs[k], np.float32).reshape(-1)
                     for k in ('n1_w', 'n2_w', 'n3_w'))
    for nb in ('n1_b', 'n2_b', 'n3_b', 'sa_qb', 'sa_kb', 'sa_vb', 'sa_pb',
               'ca_qb', 'ca_kb', 'ca_vb', 'ca_pb', 'fc1_b', 'fc2_b'):
        assert not np.any(np.asarray(inputs[nb])), f"nonzero bias {nb}"

    shared = {
        'w_sa_q': w8(ln_fold(inputs['sa_qw'], n1w, True)),
        'w_sa_k': w8(ln_fold(inputs['sa_kw'], n1w, True)),
        'w_sa_v': w8(ln_fold(inputs['sa_vw'], n1w, False)),
        'w_sa_p': w8(np.asarray(inputs['sa_pw'], np.float64).T),
        'w_ca_q': w8(ln_fold(inputs['ca_qw'], n2w, True)),
        'w_ca_k': w8(ln_fold(inputs['ca_kw'], None, True)),
        'w_ca_v': w8(np.asarray(inputs['ca_vw'], np.float64).T),
        'w_ca_p': w8(np.asarray(inputs['ca_pw'], np.float64).T),
        'w_fc1': w8(ln_fold(inputs['fc1_w'], n3w, False)),
        'w_fc2': w8(np.asarray(inputs['fc2_w'], np.float64).T),
        'gvec': np.ascontiguousarray(np.stack(
            [np.asarray(inputs['g_msa'], np.float32).reshape(-1) / SW,
             np.asarray(inputs['g_ca'], np.float32).reshape(-1) / SW,
             np.asarray(inputs['g_mlp'], np.float32).reshape(-1) / SW], 1)),
    }
    mask_T = (~am).astype(np.float32).T       # [k, q] multiplicative
    nkt = c.Lk // 128
    su_sa = [min(128 * (kt % (nkt // 2)), c.Lq - 128) for kt in range(nkt)]
    cc_np, sc_np = rope_tables32(np.arange(c.Lc), HD)
    shared['cos_c'] = to_bf(cc_np)
    shared['sin_c'] = to_bf(sc_np)

    in_maps = []
    for core in range(c.n_cores):
        b = core // c.qsplit
        qh = core % c.qsplit
        perm_tok = _core_token_perm(c, qh)
        own = perm_tok[:c.Lq]
        cq_np, sq_np = rope_tables32(own.astype(np.float64), HD)
        ck_np, sk_np = rope_tables32(perm_tok.astype(np.float64), HD)
        xb_T = x[b].T                          # [E, Lk]
        m = dict(shared)
        m.update({
            'xT': np.ascontiguousarray(xb_T[:, own]),
            'x8': to_f8(pack_pairs(np.ascontiguousarray(xb_T[:, perm_tok]))),
            'ctx8': to_f8(pack_pairs(np.ascontiguousarray(ctxv[b].T))),
            'mask8': to_f8(np.concatenate(
                [mask_T[perm_tok[kt * 128:(kt + 1) * 128]][:,
                    own[su_sa[kt]:su_sa[kt] + 128]]
                 for kt in range(nkt)], 0)),
            'cos_q': to_bf(cq_np), 'sin_q': to_bf(sq_np),
            'cos_k': to_bf(ck_np), 'sin_k': to_bf(sk_np),
        })
        in_maps.append(m)
    return in_maps


def _core_token_perm(c, qh):
    """Own (striped) 128-token tiles first, then the other range's tiles."""
    ntile = c.Lk // 128
    if c.qsplit == 1:
        order = list(range(ntile))
    else:
        own_t = list(range(qh, ntile, c.qsplit))
        oth_t = [t for t in range(ntile) if t not in own_t]
        order = own_t + oth_t
    return np.concatenate(
        [np.arange(t * 128, (t + 1) * 128) for t in order])


def assemble_output(results, cfg: Cfg):
    c = cfg
    out = np.zeros((c.B, c.LQ, c.E), np.float32)
    for core in range(c.n_cores):
        b = core // c.qsplit
        qh = core % c.qsplit
        own = _core_token_perm(c, qh)[:c.Lq]
        out[b, own, :] = np.asarray(results[core]['outT']).T
    return out


_CFG = Cfg(E=1024, H=16, LQ=1024, LC=512, B=4, FF=4096, n_cores=8)
_CACHE = {}


def kernel(**inputs):
    from concourse.bass_utils import run_bass_kernel_spmd
    cfg = _CFG
    in_maps = host_prep(inputs, cfg)
    if 'nc' not in _CACHE:
        _CACHE['nc'] = build_core_program(cfg)
    res = run_bass_kernel_spmd(_CACHE['nc'], in_maps,
                               core_ids=list(range(cfg.n_cores)))
    return assemble_output(res.results, cfg)
